# revision 1
# baseline (speedup 1.0000x reference)
"""Trainium2 Bass kernel for nn_CFDFVnewGCN (6-layer FVnewConv GNN).

Strategy: shard destination nodes (and their incoming edges) across 8 cores.
Nodes are permuted/degree-balanced into 49 windows of <=128 nodes per core.
X rows are laid out in two chunks (chunk0 = windows [0,W0), chunk1 = rest) so
each layer's AllGather is split in two and gathers depend only on the chunk
they read (chunk-relative int16 indices). All matmul/elementwise data is bf16
(PSUM accumulation f32): per 128-edge tile the scaling matmul (edge_attr
stationary, bias folded as 7th K-row) feeds an ACT relu (+DVE tail), a DVE
message multiply, and a one-hot scatter matmul accumulating aggr[window, D]
in PSUM. Per window: PE transposes of aggr + fused output matmul (bias via
const ones row), relu/tanh, DMA out.
"""
import sys
import numpy as np

for _p in ('/opt/trn_rl_repo', '/root/.axon_site/_ro/trn_rl_repo'):
    if _p not in sys.path:
        sys.path.insert(0, _p)

import concourse.bacc as bacc
import concourse.mybir as mybir
import concourse.tile as tile
from concourse.bass_utils import run_bass_kernel_spmd

import ml_dtypes

BF16NP = ml_dtypes.bfloat16
F32 = mybir.dt.float32
BF16 = mybir.dt.bfloat16
I16 = mybir.dt.int16
I32 = mybir.dt.int32
COPY = mybir.ActivationFunctionType.Copy
RELU = mybir.ActivationFunctionType.Relu
TANH = mybir.ActivationFunctionType.Tanh
MULT = mybir.AluOpType.mult
MAX = mybir.AluOpType.max
ISEQ = mybir.AluOpType.is_equal

NCORES = 8


class Cfg:
    def __init__(self, n_nodes=50000, n_edges=200000, hid=512, hs=3, ea=6,
                 out=3, w0=25):
        self.N = n_nodes
        self.E = n_edges
        self.HID = hid
        self.HS = hs
        self.EA = ea
        self.OUT = out
        self.NPC = self.N // NCORES              # nodes per core
        self.NWIN = (self.NPC + 127) // 128      # windows per core
        self.WSIZES = [128] * (self.NWIN - 1) + [self.NPC - 128 * (self.NWIN - 1)]
        # two src chunks: chunk0 = windows [0, W0), chunk1 = rest
        self.W0 = w0
        self.CWIN = [(0, w0), (w0, self.NWIN)]
        self.CROWS = [sum(self.WSIZES[a:b]) for a, b in self.CWIN]
        self.CBASE = [0, NCORES * self.CROWS[0]]
        assert NCORES * self.CROWS[0] <= 32768
        assert NCORES * self.CROWS[1] <= 32768
        self.VIEW = min(32768, self.N)      # B1 view rows [0:VIEW]
        self.B2OFS = max(0, self.N - self.VIEW)  # B2 view rows [B2OFS:N]
        # layer table
        self.LAYERS = []
        for name in ['p0', 'p1', 'p2', 'c0', 'c1', 'c2']:
            if name == 'p0':
                ic, g, oc = 7, 7, hid
            elif name == 'c0':
                ic, g, oc = hid + 4, hid, hid
            elif name == 'c2':
                ic, g, oc = hid + 1, hid, out
            else:
                ic, g, oc = hid + 1, hid, hid
            D = ic * hs
            DP = D + (D % 2)
            OCP = oc + (oc % 2)
            if oc == out:
                OCP = 4
            self.LAYERS.append(dict(name=name, ic=ic, g=g, oc=oc, D=D, DP=DP,
                                    OCP=OCP, relu=(name != 'c2')))


def _col2orig(cfg, lay):
    """Map plane-major padded column -> original scaling index j=i*HS+h, -1=pad."""
    HS, g, ic, DP = cfg.HS, lay['g'], lay['ic'], lay['DP']
    m = np.full(DP, -1, np.int64)
    if lay['name'] == 'p0':
        for h in range(HS):
            for i in range(g):
                m[h * g + i] = i * HS + h
    elif lay['name'] == 'c0':
        for h in range(HS):
            for i in range(g):
                m[h * g + i] = (3 + i) * HS + h          # fine_x at xc dims 3..
        for f in range(3):
            for h in range(HS):
                m[HS * g + 3 * f + h] = f * HS + h        # fyo
        for h in range(HS):
            m[HS * g + 9 + h] = (ic - 1) * HS + h         # na
    else:
        for h in range(HS):
            for i in range(g):
                m[h * g + i] = i * HS + h
        for h in range(HS):
            m[HS * g + h] = g * HS + h                    # na
    return m


def _balance(items_deg, caps):
    """Greedy: assign items (sorted by degree desc) to bins with capacity,
    minimizing max degree sum. Returns bin index per item."""
    order = np.argsort(-items_deg, kind='stable')
    nbins = len(caps)
    load = np.zeros(nbins)
    cnt = np.zeros(nbins, np.int64)
    out = np.zeros(len(items_deg), np.int64)
    import heapq
    heap = [(0.0, b) for b in range(nbins)]
    heapq.heapify(heap)
    for it in order:
        while True:
            l, b = heapq.heappop(heap)
            if cnt[b] < caps[b]:
                break
        out[it] = b
        cnt[b] += 1
        load[b] += items_deg[it]
        if cnt[b] < caps[b]:
            heapq.heappush(heap, (load[b], b))
    return out


def _preprocess(cfg, inputs):
    N, E, HS = cfg.N, cfg.E, cfg.HS
    ei = np.asarray(inputs['edge_index'])
    src = ei[0].astype(np.int64)
    dst = ei[1].astype(np.int64)
    deg = np.bincount(dst, minlength=N).astype(np.float64)

    node_core = _balance(deg, [cfg.NPC] * NCORES)
    node_win = np.zeros(N, np.int64)
    node_slot = np.zeros(N, np.int64)
    for c in range(NCORES):
        nodes = np.where(node_core == c)[0]
        w = _balance(deg[nodes], cfg.WSIZES)
        node_win[nodes] = w
        for wi in range(cfg.NWIN):
            sel = nodes[w == wi]
            node_slot[sel] = np.arange(len(sel))

    # x row layout: chunk-major, core-major within chunk, window-major in core
    node_row = node_win * 128 + node_slot           # within-core row (out rows)
    wchunk = np.where(np.arange(cfg.NWIN) < cfg.W0, 0, 1)
    wofs = np.zeros(cfg.NWIN, np.int64)
    for k, (a, b) in enumerate(cfg.CWIN):
        for w in range(a, b):
            wofs[w] = sum(cfg.WSIZES[a:w])
    k_of = wchunk[node_win]
    xrow = (np.array(cfg.CBASE)[k_of] + node_core * np.array(cfg.CROWS)[k_of]
            + wofs[node_win] + node_slot)
    xrow_src = xrow[src]
    src_chunk = (xrow_src >= cfg.CBASE[1]).astype(np.int64)

    # edge buckets per (core, window, src-chunk)
    ec = node_core[dst]
    ew = node_win[dst]

    # A side: strict chunk-0 (dep AG_0 only), sized so no core pads (min-floor).
    # B side: remaining edges via two flexible int16 views (dep both AGs):
    #   B1 = rows [0:VIEW), B2 = rows [B2OFS:N).
    R0 = cfg.CBASE[1]
    kA = np.zeros(cfg.NWIN, np.int64)
    kB1 = np.zeros(cfg.NWIN, np.int64)
    kB2 = np.zeros(cfg.NWIN, np.int64)
    cnt0 = np.zeros((NCORES, cfg.NWIN), np.int64)
    cntT = np.zeros((NCORES, cfg.NWIN), np.int64)
    f1 = np.zeros((NCORES, cfg.NWIN), np.int64)   # B1-only: xrow < B2OFS
    f2 = np.zeros((NCORES, cfg.NWIN), np.int64)   # B2-only: xrow >= VIEW
    np.add.at(cnt0, (ec[xrow_src < R0], ew[xrow_src < R0]), 1)
    np.add.at(cntT, (ec, ew), 1)
    np.add.at(f2, (ec[xrow_src >= cfg.VIEW], ew[xrow_src >= cfg.VIEW]), 1)
    flex0 = np.zeros((NCORES, cfg.NWIN), np.int64)  # chunk-0 in [B2OFS,R0)
    sel = (xrow_src >= cfg.B2OFS) & (xrow_src < R0)
    np.add.at(flex0, (ec[sel], ew[sel]), 1)
    for w in range(cfg.NWIN):
        kA[w] = int(cnt0[:, w].min() // 128)
        nB = cntT[:, w] - kA[w] * 128
        # forced-B1 per core: chunk-0 spills beyond the flexible pool
        sp = cnt0[:, w] - kA[w] * 128
        b1f = np.maximum(0, sp - flex0[:, w])
        kB1[w] = int(np.ceil(b1f.max() / 128))
        b2min = np.maximum(f2[:, w], nB - kB1[w] * 128)
        kB2[w] = int(np.ceil(b2min.max() / 128))
    tw = kA + kB1 + kB2
    tbase = np.concatenate([[0], np.cumsum(tw)])
    T = int(tbase[-1])

    ea_np = np.asarray(inputs['edge_attr'], np.float32)
    na_np = np.asarray(inputs['node_attr'], np.float32).reshape(-1)
    fyo_np = np.asarray(inputs['fine_y_orig'], np.float32)

    per_core = []
    for c in range(NCORES):
        ea_s = np.zeros((7, T * 128), np.float32)
        idx_s = np.zeros((16, T * 8), np.int16)
        dst_s = np.full((128, T), 999.0, np.float32)
        na_s = np.zeros((128, T), np.float32)
        fyo_s = np.zeros((128, 3 * T), np.float32)
        for w in range(cfg.NWIN):
            t0 = int(tbase[w])
            eidx = np.where((ec == c) & (ew == w))[0]
            e0 = eidx[xrow_src[eidx] < R0]
            e1 = eidx[xrow_src[eidx] >= R0]
            # A keeps chunk-0 edges; prefer spilling flexible ones (>= B2OFS)
            ord0 = np.argsort(xrow_src[e0] >= cfg.B2OFS, kind='stable')
            e0 = e0[ord0]
            nA = int(kA[w]) * 128
            A_e = e0[:min(nA, len(e0))]
            spill = e0[min(nA, len(e0)):]
            # B pool: spills + chunk-1. B1-only: xrow < B2OFS;
            # B2-only: xrow >= VIEW; else flexible.
            Bp = np.concatenate([spill, e1])
            xb = xrow_src[Bp]
            b1_only = Bp[xb < cfg.B2OFS]
            b2_only = Bp[xb >= cfg.VIEW]
            bflex = Bp[(xb >= cfg.B2OFS) & (xb < cfg.VIEW)]
            cap1 = int(kB1[w]) * 128
            take = min(len(bflex), max(0, cap1 - len(b1_only)))
            B1_e = np.concatenate([b1_only, bflex[:take]])
            B2_e = np.concatenate([b2_only, bflex[take:]])
            assert len(B1_e) <= cap1, (w, len(B1_e), cap1)
            assert len(B2_e) <= int(kB2[w]) * 128, (w, len(B2_e))
            for edges, ktiles, toff, base in (
                    (A_e, int(kA[w]), t0, 0),
                    (B1_e, int(kB1[w]), t0 + int(kA[w]), 0),
                    (B2_e, int(kB2[w]), t0 + int(kA[w]) + int(kB1[w]),
                     cfg.B2OFS)):
                nslots = ktiles * 128
                if nslots == 0:
                    continue
                assert len(edges) <= nslots
                iv = np.zeros(nslots, np.int64)
                iv[:len(edges)] = xrow_src[edges] - base
                assert iv.min() >= 0 and iv.max() < 32768
                jj = np.arange(nslots)
                idx_s[jj % 16, toff * 8 + jj // 16] = iv.astype(np.int16)
                if len(edges):
                    e_jj = jj[:len(edges)]
                    e_tt = toff + e_jj // 128
                    e_pp = e_jj % 128
                    ea_s[0:6, e_tt * 128 + e_pp] = ea_np[edges].T
                    ea_s[6, e_tt * 128 + e_pp] = 1.0
                    dst_s[e_pp, e_tt] = node_slot[dst[edges]]
                    na_s[e_pp, e_tt] = na_np[src[edges]]
                    fyo_s[e_pp.repeat(3), (e_tt * 3).repeat(3)
                          + np.tile([0, 1, 2], len(edges))] = fyo_np[src[edges]].ravel()
        per_core.append(dict(ea_s=ea_s.astype(BF16NP), idx_s=np.tile(idx_s, (8, 1)),
                             dst_s=dst_s, na_s=na_s, fyo_s=fyo_s))

    # xc0 buffer: [N, 64] f32 in x-row order: cols [x(5), sdf, na, 0...]
    # (gather elem_size must be a multiple of 256 bytes -> 64 f32 cols)
    x_np = np.asarray(inputs['x'], np.float32)
    sdf_np = np.asarray(inputs['sdf'], np.float32)
    xc0 = np.zeros((N, 64), np.float32)
    xc0[xrow, 0:5] = x_np
    xc0[xrow, 5] = sdf_np[:, 0]
    xc0[xrow, 6] = na_np

    wts = {}
    for lay in cfg.LAYERS:
        nm = lay['name']
        win = np.asarray(inputs[f'win_{nm}'], np.float32)
        bin_ = np.asarray(inputs[f'bin_{nm}'], np.float32)
        wout = np.asarray(inputs[f'wout_{nm}'], np.float32)
        bout = np.asarray(inputs[f'bout_{nm}'], np.float32)
        m = _col2orig(cfg, lay)
        DP, OCP = lay['DP'], lay['OCP']
        winT = np.zeros((7, DP), np.float32)
        sel = m >= 0
        winT[0:cfg.EA, sel] = win[m[sel]].T
        winT[6, sel] = bin_[m[sel]]
        woutT = np.zeros((DP + 1, OCP), np.float32)
        woutT[np.where(sel)[0], 0:lay['oc']] = wout[:, m[sel]].T
        woutT[DP, 0:lay['oc']] = bout
        wts[f'winT_{nm}'] = winT.astype(BF16NP)
        wts[f'woutT_{nm}'] = woutT.astype(BF16NP)

    struct = dict(kA=kA, kB1=kB1, kB2=kB2, tw=tw, tbase=tbase, T=T, TWMAX=int(tw.max()))
    asm = dict(node_core=node_core, node_row=node_row)
    return struct, per_core, wts, xc0, asm


def _build(cfg, struct, repeat=1):
    kA, kB1, kB2, tw, tbase, T = (struct['kA'], struct['kB1'], struct['kB2'],
                                  struct['tw'], struct['tbase'], struct['T'])
    TWMAX = struct['TWMAX']
    HID = cfg.HID
    R0 = cfg.CBASE[1]

    nc = bacc.Bacc("TRN2", target_bir_lowering=False, debug=False,
                   enable_asserts=True, num_devices=NCORES,
                   num_swdge_queues=4)
    ea_in = nc.dram_tensor("ea_s", [7, T * 128], BF16, kind="ExternalInput").ap()
    idx_in = nc.dram_tensor("idx_s", [128, T * 8], I16, kind="ExternalInput").ap()
    dst_in = nc.dram_tensor("dst_s", [128, T], F32, kind="ExternalInput").ap()
    na_in = nc.dram_tensor("na_s", [128, T], F32, kind="ExternalInput").ap()
    fyo_in = nc.dram_tensor("fyo_s", [128, 3 * T], F32, kind="ExternalInput").ap()
    xc0_in = nc.dram_tensor("xc0_in", [cfg.N, 64], F32, kind="ExternalInput").ap()
    win_ins = {}
    wout_ins = {}
    for lay in cfg.LAYERS:
        nm = lay['name']
        win_ins[nm] = nc.dram_tensor(f"winT_{nm}", [7, lay['DP']], BF16,
                                     kind="ExternalInput").ap()
        wout_ins[nm] = nc.dram_tensor(f"woutT_{nm}", [lay['DP'] + 1, lay['OCP']],
                                      BF16, kind="ExternalInput").ap()
    out_fin = nc.dram_tensor("out_final", [cfg.NPC, cfg.OUT], F32,
                             kind="ExternalOutput").ap()

    DPMAX = max(l['DP'] for l in cfg.LAYERS)
    NFULLMAX = max(l['DP'] // 128 for l in cfg.LAYERS)
    TAILMAX = max(l['DP'] - 128 * (l['DP'] // 128) for l in cfg.LAYERS)
    OCPMAX = max(l['OCP'] for l in cfg.LAYERS)

    with tile.TileContext(nc) as tc:
        with (
            tc.tile_pool(name="cst", bufs=1) as cst,
            tc.tile_pool(name="sbw", bufs=2) as sbw,
            tc.tile_pool(name="gst", bufs=3) as gst,
            tc.tile_pool(name="eap", bufs=3) as eap,
            tc.tile_pool(name="scp", bufs=3) as scp,
            tc.tile_pool(name="msgp", bufs=3) as msgp,
            tc.tile_pool(name="Sp", bufs=3) as Sp,
            tc.tile_pool(name="agsp", bufs=3) as agsp,
            tc.tile_pool(name="agtp", bufs=3) as agtp,
            tc.tile_pool(name="outp", bufs=2) as outp,
            tc.tile_pool(name="ps_sc", bufs=2, space="PSUM") as ps_sc,
            tc.tile_pool(name="ps_ag", bufs=1, space="PSUM") as ps_ag,
            tc.tile_pool(name="ps_tp", bufs=1, space="PSUM") as ps_tp,
            tc.tile_pool(name="ps_om", bufs=2, space="PSUM") as ps_om,
            tc.tile_pool(name="dram", bufs=1, space="DRAM") as dram,
        ):
            # ---- constants
            iota_i = cst.tile([128, 128], I32)
            nc.gpsimd.iota(iota_i[:, :], pattern=[[1, 128]], base=0,
                           channel_multiplier=0)
            iota_bf = cst.tile([128, 128], BF16)
            nc.vector.tensor_copy(iota_bf[:, :], iota_i[:, :])
            iota_p = cst.tile([128, 1], I32)
            nc.gpsimd.iota(iota_p[:, :], pattern=[[1, 1]], base=0,
                           channel_multiplier=1)
            iota_pf = cst.tile([128, 1], F32)
            nc.vector.tensor_copy(iota_pf[:, :], iota_p[:, :])
            ident = cst.tile([128, 128], BF16)
            nc.vector.tensor_scalar(out=ident[:, :], in0=iota_bf[:, :],
                                    scalar1=iota_pf[:, :], scalar2=None,
                                    op0=ISEQ)
            ones_i = cst.tile([1, 128], I32)
            nc.gpsimd.iota(ones_i[:, :], pattern=[[0, 128]], base=1,
                           channel_multiplier=0)
            ones = cst.tile([1, 128], BF16)
            nc.vector.tensor_copy(ones[:, :], ones_i[:, :])

            # ---- static per-slot data (resident)
            dst_sb = cst.tile([128, T], F32)
            nc.sync.dma_start(out=dst_sb[:, :], in_=dst_in[:, :])
            na_sb = cst.tile([128, T], F32)
            nc.sync.dma_start(out=na_sb[:, :], in_=na_in[:, :])
            fyo_sb = cst.tile([128, 3 * T], F32)
            nc.sync.dma_start(out=fyo_sb[:, :], in_=fyo_in[:, :])
            idx_sb = cst.tile([128, T * 8], I16)
            nc.sync.dma_start(out=idx_sb[:, :], in_=idx_in[:, :])

            # ---- DRAM buffers
            xc0b = dram.tile([cfg.N, 64], F32)
            nc.sync.dma_start(out=xc0b[:, :], in_=xc0_in[:, :])
            xstate = {}

            def emit_layer(lay, rep):
                nm, DP, OCP, g = lay['name'], lay['DP'], lay['OCP'], lay['g']
                nfull = DP // 128
                tailk = DP - nfull * 128
                tail_lo = 1536 if DP > 1536 else 0
                tailk2 = DP - tail_lo
                pieces = [(i, min(i + 512, tail_lo))
                          for i in range(0, tail_lo, 512)]
                if nm == 'p0':
                    gsrc, gcols = xc0b, 64
                else:
                    gsrc, gcols = xstate['cur'], HID

                # layer weights (bf16 direct)
                winT = sbw.tile([7, DPMAX], BF16, tag="winT")
                nc.sync.dma_start(out=winT[:, 0:DP], in_=win_ins[nm][:, :])
                wt = sbw.tile([128, NFULLMAX * OCPMAX], BF16, tag="wt")
                for ci in range(nfull):
                    nc.sync.dma_start(
                        out=wt[:, ci * OCP:ci * OCP + OCP],
                        in_=wout_ins[nm][ci * 128:(ci + 1) * 128, :])
                wtail = sbw.tile([TAILMAX, OCPMAX], BF16, tag="wtail")
                if tailk:
                    nc.sync.dma_start(
                        out=wtail[0:tailk, 0:OCP],
                        in_=wout_ins[nm][nfull * 128:nfull * 128 + tailk, :])
                wbias = sbw.tile([1, OCPMAX], BF16, tag="wbias")
                nc.sync.dma_start(out=wbias[:, 0:OCP],
                                  in_=wout_ins[nm][DP:DP + 1, :])

                if nm != 'c2':
                    Xout = dram.tile([cfg.N, HID], BF16,
                                     tag=f"X_{nm}", name=f"X_{nm}_{rep}")
                    xstate['cur'] = Xout
                    oslices = []
                    for k, r in enumerate(cfg.CROWS):
                        t_ = dram.tile([r, HID], BF16, tag=f"osl_{nm}_{k}",
                                       name=f"osl_{nm}_{k}_{rep}")
                        oslices.append(t_)

                xdt_l = F32 if nm == 'p0' else BF16

                def tile_compute(ear, tg, te, xst):
                    sc = scp.tile([128, DPMAX], BF16, tag="sc")
                    sc_pieces = pieces + ([(tail_lo, DP)] if tailk2 else [])
                    for pi, (p0_, p1_) in enumerate(sc_pieces):
                        scps = ps_sc.tile([128, 512], F32, tag="scps")
                        nc.tensor.matmul(
                            scps[:, 0:p1_ - p0_],
                            ear[:, te * 128:(te + 1) * 128],
                            winT[:, p0_:p1_], start=True, stop=True)
                        if p1_ - p0_ > 64 and (pi < 2 or te % 2 == 0):
                            nc.scalar.activation(sc[:, p0_:p1_],
                                                 scps[:, 0:p1_ - p0_], RELU)
                        else:
                            nc.vector.tensor_scalar(
                                out=sc[:, p0_:p1_],
                                in0=scps[:, 0:p1_ - p0_],
                                scalar1=0.0, scalar2=None, op0=MAX)
                    msg = msgp.tile([128, DPMAX], BF16, tag="msg")
                    if nm == 'p0':
                        for h in range(cfg.HS):
                            lo = h * g
                            hi = lo + g + (1 if h == cfg.HS - 1 else 0)
                            nc.vector.tensor_tensor(
                                out=msg[:, lo:hi], in0=sc[:, lo:hi],
                                in1=xst[:, 0:hi - lo], op=MULT)
                    else:
                        for h in range(cfg.HS):
                            nc.vector.tensor_tensor(
                                out=msg[:, h * g:(h + 1) * g],
                                in0=sc[:, h * g:(h + 1) * g],
                                in1=xst[:, 0:g], op=MULT)
                        nab = cfg.HS * g + (9 if nm == 'c0' else 0)
                        nc.vector.tensor_tensor(
                            out=msg[:, nab:DP], in0=sc[:, nab:DP],
                            in1=na_sb[:, tg:tg + 1].broadcast_to(
                                [128, DP - nab]), op=MULT)
                        if nm == 'c0':
                            for f in range(3):
                                lo = cfg.HS * g + 3 * f
                                nc.vector.tensor_tensor(
                                    out=msg[:, lo:lo + 3],
                                    in0=sc[:, lo:lo + 3],
                                    in1=fyo_sb[:, 3 * tg + f:3 * tg + f + 1]
                                    .broadcast_to([128, 3]), op=MULT)
                    S = Sp.tile([128, 128], BF16, tag="S")
                    nc.vector.tensor_scalar(
                        out=S[:, :], in0=iota_bf[:, :],
                        scalar1=dst_sb[:, tg:tg + 1], scalar2=None,
                        op0=ISEQ)
                    return msg, S

                def tile_scatter(agps, agt2, msg, S, start, stop):
                    for (p0_, p1_) in pieces:
                        nc.tensor.matmul(
                            agps[:, p0_:p1_], S[:, :], msg[:, p0_:p1_],
                            start=start, stop=stop)
                    if tailk2:
                        nc.tensor.matmul(
                            agt2[0:tailk2, :], msg[:, tail_lo:DP], S[:, :],
                            start=start, stop=stop)

                for w in range(cfg.NWIN):
                    nt = int(tw[w])
                    t0 = int(tbase[w])
                    ka, kb1, kb2 = int(kA[w]), int(kB1[w]), int(kB2[w])
                    kb = kb1 + kb2
                    wsz = cfg.WSIZES[w]
                    wchunk = 0 if w < cfg.W0 else 1
                    # gathers: A (chunk-0 rows, dep AG_0 only) into xstA;
                    # B1/B2 (flexible int16 views, dep both AGs) into xstB
                    xstA = gst.tile([128, TWMAX, gcols], xdt_l,
                                    tag=f"xstA_{nm == 'p0'}")
                    xstB = gst.tile([128, TWMAX, gcols], xdt_l,
                                    tag=f"xstB_{nm == 'p0'}")
                    if ka:
                        nc.gpsimd.dma_gather(
                            out_ap=xstA[:, 0:ka, 0:gcols],
                            in_ap=gsrc[0:R0, :],
                            idxs_ap=idx_sb[:, t0 * 8:(t0 + ka) * 8],
                            num_idxs=ka * 128, num_idxs_reg=ka * 128,
                            elem_size=gcols, queue_num=0)
                    if kb1:
                        nc.gpsimd.dma_gather(
                            out_ap=xstB[:, 0:kb1, 0:gcols],
                            in_ap=gsrc[0:cfg.VIEW, :],
                            idxs_ap=idx_sb[:, (t0 + ka) * 8:(t0 + ka + kb1) * 8],
                            num_idxs=kb1 * 128, num_idxs_reg=kb1 * 128,
                            elem_size=gcols, queue_num=0)
                    if kb2:
                        nc.gpsimd.dma_gather(
                            out_ap=xstB[:, kb1:kb, 0:gcols],
                            in_ap=gsrc[cfg.B2OFS:cfg.N, :],
                            idxs_ap=idx_sb[:, (t0 + ka + kb1) * 8:(t0 + nt) * 8],
                            num_idxs=kb2 * 128, num_idxs_reg=kb2 * 128,
                            elem_size=gcols, queue_num=0)
                    ear = eap.tile([7, TWMAX * 128], BF16, tag="ear")
                    nc.sync.dma_start(out=ear[:, 0:nt * 128],
                                      in_=ea_in[:, t0 * 128:(t0 + nt) * 128])

                    agps = ps_ag.tile([128, 1536], F32, tag="agps")
                    agt2 = ps_tp.tile([TAILMAX, 128], F32, tag="agt2")
                    prev = None
                    for t in range(nt):
                        xst = (xstA[:, t, :] if t < ka
                               else xstB[:, t - ka, :])
                        msgS = tile_compute(ear, t0 + t, t, xst)
                        if prev is not None:
                            tile_scatter(agps, agt2, *prev, start=(t == 1),
                                         stop=False)
                        prev = msgS
                    tile_scatter(agps, agt2, *prev, start=(nt == 1), stop=True)
                    # evacuate aggr: split halves across ACT and DVE; the tail
                    # was scattered directly transposed (agt2)
                    ags = agsp.tile([128, DPMAX], BF16, tag="ags")
                    agT = agtp.tile([128, NFULLMAX, 128], BF16, tag="agT")
                    agTt = agtp.tile([TAILMAX, 128], BF16, tag="agTt")
                    if tail_lo:
                        h0 = 768
                        nc.scalar.activation(ags[:, 0:h0], agps[:, 0:h0],
                                             COPY)
                        nc.vector.tensor_copy(ags[:, h0:tail_lo],
                                              agps[:, h0:tail_lo])
                        nc.sync.dma_start_transpose(
                            out=agT[:, 0:nfull, :],
                            in_=ags[:, 0:nfull * 128])
                    nc.vector.tensor_copy(agTt[0:tailk2, :],
                                          agt2[0:tailk2, :])
                    # out matmul
                    om = ps_om.tile([128, OCPMAX], F32, tag="om")
                    for ci in range(nfull):
                        nc.tensor.matmul(
                            om[:, 0:OCP],
                            agT[:, ci, :],
                            wt[:, ci * OCP:(ci + 1) * OCP],
                            start=(ci == 0), stop=False)
                    if tailk2:
                        nc.tensor.matmul(om[:, 0:OCP], agTt[0:tailk2, :],
                                         wtail[0:tailk2, 0:OCP],
                                         start=(nfull == 0), stop=False)
                    nc.tensor.matmul(om[:, 0:OCP], ones[:, :],
                                     wbias[:, 0:OCP], start=False, stop=True)
                    if lay['relu']:
                        outs = outp.tile([128, OCPMAX], BF16, tag="outs")
                        omr = outp.tile([128, OCPMAX], BF16, tag="omr")
                        nc.scalar.activation(omr[:, 0:OCP], om[:, 0:OCP], TANH)
                        nc.vector.tensor_scalar(
                            out=outs[:, 0:OCP], in0=omr[:, 0:OCP],
                            scalar1=0.0, scalar2=None, op0=MAX)
                        ro = wofs_of(cfg, w)
                        nc.sync.dma_start(
                            out=oslices[wchunk][ro:ro + wsz, :],
                            in_=outs[0:wsz, 0:HID])
                    else:
                        outs = outp.tile([128, OCPMAX], F32, tag="outsf")
                        nc.scalar.activation(outs[:, 0:OCP], om[:, 0:OCP],
                                             TANH)
                        nc.sync.dma_start(
                            out=out_fin[w * 128:w * 128 + wsz, :],
                            in_=outs[0:wsz, 0:cfg.OUT])
                    # chunk AllGathers (emitted a few windows late so the
                    # collective's wait doesn't stall the in-order Pool queue)
                    if nm != 'c2' and w == min(cfg.W0 + 3, cfg.NWIN - 2):
                        nc.gpsimd.collective_compute(
                            "AllGather", mybir.AluOpType.bypass,
                            replica_groups=[list(range(NCORES))],
                            ins=[oslices[0][:, :]],
                            outs=[Xout[0:R0, :]])
                if nm != 'c2':
                    nc.gpsimd.collective_compute(
                        "AllGather", mybir.AluOpType.bypass,
                        replica_groups=[list(range(NCORES))],
                        ins=[oslices[1][:, :]],
                        outs=[Xout[R0:cfg.N, :]])

            for rep in range(repeat):
                for lay in cfg.LAYERS:
                    emit_layer(lay, rep)
    nc.compile()
    return nc


def wofs_of(cfg, w):
    a, b = (0, cfg.W0) if w < cfg.W0 else (cfg.W0, cfg.NWIN)
    return sum(cfg.WSIZES[a:w])


def _run(inputs, trace=False, repeat=1):
    cfg = Cfg()
    struct, per_core, wts, xc0, asm = _preprocess(cfg, inputs)
    nc = _build(cfg, struct, repeat=repeat)
    in_maps = []
    for c in range(NCORES):
        im = dict(per_core[c])
        im['xc0_in'] = xc0
        for k, v in wts.items():
            im[k] = v
        in_maps.append(im)
    res = run_bass_kernel_spmd(nc, in_maps, list(range(NCORES)), trace=trace)
    out = np.zeros((cfg.N, cfg.OUT), np.float32)
    for c in range(NCORES):
        sl = res.results[c]['out_final']
        sel = asm['node_core'] == c
        out[sel] = sl[asm['node_row'][sel]]
    return out, res


def kernel(**inputs):
    return _run(inputs, trace=False)[0]

